# revision 18
# baseline (speedup 1.0000x reference)
"""Trainium2 Bass kernel for nn_CrossModalAttention (B=2, LQ=LK=2048,
QDIM=HID=1024, KDIM=VDIM=768, H=16, D=64).

Sharding: 8 cores = 2 batches x 4 head-groups (4 heads each).
Per core: q/k/v projections column-sliced over HID, attention for its 4
heads, row-parallel partial of the out-projection. Host sums the 4
partials per batch (the row-parallel unshard) and adds bo.

Device dataflow (per core), matmuls in bf16 (~4e-3 rel):
  - host passes query/key/value[b] transposed and K/V key-compacted
    (query_mask masks the KEY axis globally per batch); all DRAM
    layouts are arranged so every DMA slice is contiguous per
    partition (few descriptors -> cheap on the sync queue)
  - a short dummy-matmul warmup stream runs during the input DMAs so
    the PE HAM clock-gate is already at 8/8 when real work arrives
  - per head pair (row-packed K=64 matmuls via tile_position):
    scoresT [keys, q] -> ACT exp(s/8 + mask_bias) -> PV matmul with a
    ones-augmented V (M=65) giving ctxT and the softmax denominator
  - normalize: evac ctx PSUM (DVE/gpsimd split), reciprocal of the
    denominator row, gpsimd partition-broadcast, one DVE multiply
  - out-projection is interleaved per token-block into the next
    block's attention (kk-major so the ctx stationary is reused);
    bf16 partials stream to DRAM throughout the kernel
"""

import math

import ml_dtypes
import numpy as np

B, LQ, LK = 2, 2048, 2048
QDIM, KDIM, VDIM, HID, H = 1024, 768, 768, 1024, 16
D = HID // H  # 64
HG = 4  # head-groups (cores per batch)
HL = H // HG  # heads per core = 4
GH = HL * D  # per-core hid slice = 256
N_CORES = 8
TB = 512  # token block
NTB = LQ // TB  # 4
NEG = -1.0e30

BF16 = True
INTERLEAVE = True
OLD_NORM = False
MUL_GP = False
WARMUP = True
PROFILE = False
LAST_EXEC_NS = None
LAST_TRACE_DIR = None

_CACHE = {}
_BATCH_CACHE = {}


def _build(nkt: int, with_bv: bool, bf16: bool):
    import concourse.bacc as bacc
    import concourse.mybir as mybir
    import concourse.tile as tile

    nkeys = nkt * 128
    nkb = (nkeys + 511) // 512  # 512-key blocks
    kbs = [min(512, nkeys - kb * 512) for kb in range(nkb)]

    f32 = mybir.dt.float32
    f32r = mybir.dt.bfloat16 if bf16 else mybir.dt.float32r
    odt = mybir.dt.bfloat16 if bf16 else f32
    Exp = mybir.ActivationFunctionType.Exp

    nc = bacc.Bacc(
        "TRN2", target_bir_lowering=False, debug=False, num_devices=N_CORES
    )

    # DRAM tensors (per-core shapes); layouts chosen so each DMA slice is
    # contiguous per partition
    XQ = nc.dram_tensor("xq", [128, NTB, 8, TB], f32r, kind="ExternalInput").ap()
    XK = nc.dram_tensor("xk", [128, nkb, 6, 512], f32r, kind="ExternalInput").ap()
    XV = nc.dram_tensor("xv", [128, nkb, 6, 512], f32r, kind="ExternalInput").ap()
    WQ = nc.dram_tensor("wq", [128, 8, GH], f32r, kind="ExternalInput").ap()
    WK = nc.dram_tensor("wk", [128, 6, GH], f32r, kind="ExternalInput").ap()
    WV = nc.dram_tensor("wv", [128, 6, GH], f32r, kind="ExternalInput").ap()
    WO = nc.dram_tensor("wo", [128, 2, QDIM], f32r, kind="ExternalInput").ap()
    MB = nc.dram_tensor("mbias", [128, nkt], f32, kind="ExternalInput").ap()
    BQ = nc.dram_tensor("bqk", [128, 4], f32, kind="ExternalInput").ap()
    BV = None
    if with_bv:
        BV = nc.dram_tensor("bv", [128, 2], f32, kind="ExternalInput").ap()
    OUT = nc.dram_tensor("outp", [LQ, QDIM], odt, kind="ExternalOutput").ap()

    with tile.TileContext(nc) as tc:
        with (
            tc.tile_pool(name="consts", bufs=1) as consts,
            tc.tile_pool(name="resid", bufs=1) as resid,
            tc.tile_pool(name="xs", bufs=2) as xs,
            tc.tile_pool(name="probs", bufs=4) as probs_pool,
            tc.tile_pool(name="norm", bufs=3) as norm_pool,
            tc.tile_pool(name="outs", bufs=4) as outs_pool,
            tc.tile_pool(name="ps", bufs=2, space="PSUM") as ps,
        ):
            # ---- PE warmup: dummy matmuls issued before any data lands so
            # the HAM clock-gate reaches 8/8 while the input DMAs stream ----
            wu = consts.tile([128, 512], f32r)
            nc.vector.memset(wu, 0.0)
            wu_ps2 = None
            if WARMUP:
                wu_ps = ps.tile([128, 512], f32, tag="proj", name="warm")
                wu_ps2 = ps.tile([128, 512], f32, tag="proj", name="warm2")
                for _ in range(12):
                    nc.tensor.matmul(wu_ps, wu[:, 0:128], wu, start=True, stop=True)

            # ---- constants / weights ----
            # weights go on the gpsimd SWDGE ring so the big input streams
            # (sync HWDGE ring) aren't serialized behind them
            wq_sb = consts.tile([128, 8, GH], f32r)
            wk_sb = consts.tile([128, 6, GH], f32r)
            wv_sb = consts.tile([128, 6, GH], f32r)
            wo_sb = consts.tile([128, 2, QDIM], f32r)
            mb_sb = consts.tile([128, nkt], f32)
            bqk_sb = consts.tile([128, 4], f32)
            nc.gpsimd.dma_start(out=wq_sb, in_=WQ)
            nc.scalar.dma_start(out=bqk_sb, in_=BQ)
            nc.scalar.dma_start(out=mb_sb, in_=MB)
            nc.gpsimd.dma_start(out=wk_sb, in_=WK)
            nc.gpsimd.dma_start(out=wv_sb, in_=WV)
            nc.gpsimd.dma_start(out=wo_sb, in_=WO)
            bv_sb = None
            if with_bv:
                bv_sb = consts.tile([128, 2], f32)
                nc.gpsimd.dma_start(out=bv_sb, in_=BV)

            # ---- residents ----
            # qT tiles double as ctxT tiles later (WAR handled by Tile)
            qT = [resid.tile([128, LQ], f32r, tag=f"qT{p}", name=f"qT{p}") for p in range(2)]
            kT = [resid.tile([128, nkeys], f32r, tag=f"kT{p}", name=f"kT{p}") for p in range(2)]
            v_sb = resid.tile([128, nkt, HL, D + 1], f32r)
            # ones columns for the denominator rows: fill the whole tile,
            # the v-projection copies then overwrite the [., ., ., 0:D] part
            if bf16:
                nc.vector.memset(v_sb, 1.0)
            else:
                nc.vector.memset(v_sb[:, :, :, :].bitcast(f32), 1.0)

            # ---- k projection (per key-block, so attention can begin
            # after kb0) ----
            def emit_xk(kb_i):
                xk_t = xs.tile([128, 6, 512], f32r, tag="xk", name="xk_t", bufs=3)
                nc.sync.dma_start(out=xk_t, in_=XK[:, kb_i])
                return xk_t

            def emit_kproj_m(kb_i, m, xk_t):
                kbw = kbs[kb_i]
                s0 = kb_i * 512
                ps_t = ps.tile([128, 512], f32, tag="proj", name="kp_ps")
                for k in range(6):
                    nc.tensor.matmul(
                        ps_t[:, :kbw],
                        wk_sb[:, k, m * 128 : (m + 1) * 128],
                        xk_t[:, k, :kbw],
                        start=(k == 0),
                        stop=(k == 5),
                    )
                nc.vector.tensor_scalar_add(
                    kT[m][:, s0 : s0 + kbw],
                    ps_t[:, :kbw],
                    bqk_sb[:, 2 + m : 3 + m],
                )

            # ---- v projection ----
            vproj_state = {}

            def emit_vproj_kt(kt):
                kb_i = kt // 4
                sub = kt % 4
                if sub == 0 and kb_i not in vproj_state:
                    xv_t = xs.tile([128, 6, 512], f32r, tag="xv", name="xv_t")
                    nc.scalar.dma_start(out=xv_t, in_=XV[:, kb_i])
                    vproj_state[kb_i] = xv_t
                xv_t = vproj_state[kb_i]
                ps_t = ps.tile([128, 512], f32, tag="proj", name="vp_ps")
                for k in range(6):
                    nc.tensor.matmul(
                        ps_t[:, :GH],
                        xv_t[:, k, sub * 128 : (sub + 1) * 128],
                        wv_sb[:, k, :],
                        start=(k == 0),
                        stop=(k == 5),
                    )
                nc.vector.tensor_copy(
                    v_sb[:, kt, :, 0:D],
                    ps_t[:, :GH].rearrange("p (h d) -> p h d", h=HL),
                )

            def emit_xq(tb, eng=None):
                xq_t = xs.tile([128, 8, TB], f32r, tag="xq", name="xq_t")
                (eng or nc.sync).dma_start(out=xq_t, in_=XQ[:, tb])
                return xq_t

            def emit_qproj_m(tb, m, xq_t):
                t0 = tb * TB
                ps_t = ps.tile([128, 512], f32, tag="proj", name="qp_ps")
                for k in range(8):
                    nc.tensor.matmul(
                        ps_t,
                        wq_sb[:, k, m * 128 : (m + 1) * 128],
                        xq_t[:, k, :],
                        start=(k == 0),
                        stop=(k == 7),
                    )
                nc.vector.tensor_scalar_add(
                    qT[m][:, t0 : t0 + TB],
                    ps_t,
                    bqk_sb[:, m : m + 1],
                )

            def emit_scores(p, tb, kt, prtag, prbufs):
                t0 = tb * TB
                k0 = kt * 128
                sc = ps.tile([128, 2, TB], f32, tag="sc", name="sc")
                for hh in range(2):
                    nc.tensor.matmul(
                        sc[:, hh, :],
                        kT[p][hh * 64 : hh * 64 + 64, k0 : k0 + 128],
                        qT[p][hh * 64 : hh * 64 + 64, t0 : t0 + TB],
                        start=True,
                        stop=True,
                        tile_position=(hh * 64, 0),
                    )
                pr = probs_pool.tile(
                    [128, 2, TB], f32r, tag=prtag, name="pr", bufs=prbufs
                )
                nc.scalar.activation(
                    pr, sc, Exp, bias=mb_sb[:, kt : kt + 1], scale=0.125
                )
                return pr

            def emit_pv(p, tb, kt, pr, ctx_ps):
                for hh in range(2):
                    nc.tensor.matmul(
                        ctx_ps[hh],
                        v_sb[:, kt, 2 * p + hh, :],
                        pr[:, hh, :],
                        start=(kt == 0),
                        stop=(kt == nkt - 1),
                    )

            def emit_normalize(p, tb, ctx_ps, last=False):
                t0 = tb * TB

                def finish(hh, src, mul_eng):
                    dcp = norm_pool.tile([1, TB], f32, tag="dcp", name="dcp")
                    nc.vector.tensor_copy(dcp, src[D : D + 1, :])
                    rec = norm_pool.tile([1, TB], f32, tag="rec", name="rec")
                    nc.vector.reciprocal_approx_fast(out=rec, in_=dcp)
                    rbc = norm_pool.tile([D, TB], f32, tag="rbc", name="rbc")
                    nc.gpsimd.partition_broadcast(rbc, rec)
                    dst = qT[p][hh * 64 : hh * 64 + 64, t0 : t0 + TB]
                    mul_eng.tensor_mul(dst, src[0:D, :], rbc)
                    if with_bv:
                        nc.vector.tensor_scalar_add(
                            dst, dst, bv_sb[64 * hh : 64 * hh + 64, p : p + 1]
                        )

                if OLD_NORM:
                    evac = []
                    for hh in range(2):
                        ctmp = norm_pool.tile([D, TB], f32, tag="ctmp", name="ctmp")
                        nc.vector.tensor_copy(ctmp, ctx_ps[hh][0:D, :])
                        dcp = norm_pool.tile([1, TB], f32, tag="dcp", name="dcp")
                        nc.vector.tensor_copy(dcp, ctx_ps[hh][D : D + 1, :])
                        evac.append((ctmp, dcp))
                    for hh in range(2):
                        ctmp, dcp = evac[hh]
                        rbc = norm_pool.tile([D, TB], f32, tag="rbc", name="rbc")
                        nc.gpsimd.partition_broadcast(rbc, dcp)
                        rec = norm_pool.tile([D, TB], f32, tag="rec2", name="rec2")
                        nc.vector.reciprocal_approx_fast(out=rec, in_=rbc)
                        dst = qT[p][hh * 64 : hh * 64 + 64, tb * TB : (tb + 1) * TB]
                        nc.vector.tensor_mul(dst, ctmp, rec)
                        if with_bv:
                            nc.vector.tensor_scalar_add(
                                dst, dst, bv_sb[64 * hh : 64 * hh + 64, p : p + 1]
                            )
                elif last:
                    for hh in range(2):
                        finish(hh, ctx_ps[hh], nc.vector)
                else:
                    # evac both ctx tiles first so the next block's PV
                    # matmuls get their PSUM slots fast; the multiplies are
                    # SBUF-only and go to gpsimd to keep DVE light
                    evs = []
                    for hh in range(2):
                        ev = norm_pool.tile(
                            [D + 1, TB], f32, tag=f"ev{hh}", name="ev"
                        )
                        nc.vector.tensor_copy(ev, ctx_ps[hh])
                        evs.append(ev)
                    for hh in range(2):
                        finish(hh, evs[hh], nc.gpsimd if MUL_GP else nc.vector)

            def emit_attn(p, tb, thunks=(), last=False):
                thunks = list(thunks)
                ctx_ps = [
                    ps.tile([D + 1, TB], f32, tag="ctx", name=f"ctx{p}_{tb}_{i}")
                    for i in range(2)
                ]
                prs = [emit_scores(p, tb, 0, "pr", 4)]
                for kt in range(nkt):
                    if kt + 1 < nkt:
                        prs.append(emit_scores(p, tb, kt + 1, "pr", 4))
                    emit_pv(p, tb, kt, prs[kt], ctx_ps)
                    if kt < len(thunks):
                        thunks[kt]()
                for th in thunks[nkt:]:
                    th()
                emit_normalize(p, tb, ctx_ps, last=last)

            def qproj_thunks(tb):
                # 7 small thunks: the xq DMA, then 3+3+2 matmuls per
                # m-half chained into one psum accumulation
                cell = {}

                def start(tb=tb):
                    cell["xq"] = emit_xq(tb, nc.scalar if tb == 1 else nc.sync)

                def chunk(m, k0, k1, fin, tb=tb):
                    if k0 == 0:
                        cell[m] = ps.tile(
                            [128, 512], f32, tag="proj", name="qp_ps"
                        )
                    ps_t = cell[m]
                    for k in range(k0, k1):
                        nc.tensor.matmul(
                            ps_t,
                            wq_sb[:, k, m * 128 : (m + 1) * 128],
                            cell["xq"][:, k, :],
                            start=(k == 0),
                            stop=(k == 7),
                        )
                    if fin:
                        nc.vector.tensor_scalar_add(
                            qT[m][:, tb * TB : (tb + 1) * TB],
                            ps_t,
                            bqk_sb[:, m : m + 1],
                        )

                out = [start]
                for m in range(2):
                    out += [
                        lambda m=m: chunk(m, 0, 3, False),
                        lambda m=m: chunk(m, 3, 6, False),
                        lambda m=m: chunk(m, 6, 8, True),
                    ]
                return out

            def emit_outproj_tt(tt):
                # kk-major: one ctx stationary load feeds both nh halves
                ps0 = ps.tile([128, 512], f32, tag="proj", name="op0")
                ps1 = ps.tile([128, 512], f32, tag="proj", name="op1")
                for kk in range(2):
                    lhs = qT[kk][:, tt * 128 : (tt + 1) * 128]
                    nc.tensor.matmul(
                        ps0, lhs, wo_sb[:, kk, 0:512],
                        start=(kk == 0), stop=(kk == 1),
                    )
                    nc.tensor.matmul(
                        ps1, lhs, wo_sb[:, kk, 512:1024],
                        start=(kk == 0), stop=(kk == 1),
                    )
                for nh, pst in ((0, ps0), (1, ps1)):
                    o_sb = outs_pool.tile([128, 512], odt, tag="osb", name="o_sb")
                    nc.vector.tensor_copy(o_sb, pst)
                    oeng = nc.gpsimd if (tt + nh) % 2 == 0 else nc.sync
                    oeng.dma_start(
                        out=OUT[
                            tt * 128 : (tt + 1) * 128,
                            nh * 512 : (nh + 1) * 512,
                        ],
                        in_=o_sb,
                    )

            def outproj_thunks(tb):
                return [
                    (lambda tt=tt: emit_outproj_tt(tt))
                    for tt in range(4 * tb, 4 * tb + 4)
                ]

            # ---- emission schedule ----
            # sync-queue order = need order: xk0, xq0, xk1, xk2, ...
            xk_ts = [emit_xk(0)]
            xq0 = emit_xq(0)
            xk_ts += [emit_xk(kb_i) for kb_i in range(1, nkb)]
            emit_kproj_m(0, 0, xk_ts[0])
            emit_kproj_m(0, 1, xk_ts[0])
            emit_qproj_m(0, 0, xq0)
            emit_qproj_m(0, 1, xq0)
            kt_kb0 = min(4, nkt)
            prs0 = [emit_scores(0, 0, kt, "pr0", nkt) for kt in range(kt_kb0)]
            for kb_i in range(1, nkb):
                emit_kproj_m(kb_i, 0, xk_ts[kb_i])
                emit_kproj_m(kb_i, 1, xk_ts[kb_i])
            prs0 += [
                emit_scores(0, 0, kt, "pr0", nkt) for kt in range(kt_kb0, nkt)
            ]
            for kt in range(nkt):
                emit_vproj_kt(kt)
            ctx0 = [
                ps.tile([D + 1, TB], f32, tag="ctx", name=f"ctx00_{i}")
                for i in range(2)
            ]
            for kt in range(nkt):
                emit_pv(0, 0, kt, prs0[kt], ctx0)
            emit_normalize(0, 0, ctx0)
            if INTERLEAVE:
                emit_attn(1, 0, thunks=qproj_thunks(1))
                for tb in range(1, NTB):
                    op = outproj_thunks(tb - 1)
                    emit_attn(0, tb, thunks=op[:2])
                    th = op[2:]
                    if tb + 1 < NTB:
                        th = th + qproj_thunks(tb + 1)
                    emit_attn(1, tb, thunks=th, last=(tb == NTB - 1))
                for _ in range(10):
                    nc.tensor.matmul(
                        wu_ps2, wu[:, 0:128], wu, start=True, stop=True
                    )
                for tt in range(4 * (NTB - 1), 4 * NTB):
                    emit_outproj_tt(tt)
            else:
                ctx1b = [
                    ps.tile([D + 1, TB], f32, tag="ctx", name=f"ctx10b_{i}")
                    for i in range(2)
                ]
                for kt in range(nkt):
                    emit_pv(1, 0, kt, prs1[kt], ctx1b)
                emit_normalize(1, 0, ctx1b)
                for tb in range(1, NTB):
                    xq_t = emit_xq(tb)
                    emit_qproj_m(tb, 0, xq_t)
                    emit_attn(0, tb)
                    emit_qproj_m(tb, 1, xq_t)
                    emit_attn(1, tb)
                for tt in range(4 * NTB):
                    emit_outproj_tt(tt)

    nc.compile()
    return nc


def _prep_batch(b, query, key, value, qm, nkt, nkb, wdt):
    """Per-batch input arrays (shared by the 4 cores of a batch)."""
    nkeys = nkt * 128
    nkp = nkb * 512
    idx = np.flatnonzero(qm[b] != 0)

    def kmajor(a, ktiles):  # [dim, n] -> [128, ktiles, n]
        return np.ascontiguousarray(
            a.reshape(ktiles, 128, a.shape[1]).transpose(1, 0, 2)
        )

    xq = kmajor(np.ascontiguousarray(query[b].T).astype(wdt), 8)
    # [128, 8, LQ] -> [128, NTB, 8, TB]
    xq = np.ascontiguousarray(
        xq.reshape(128, 8, NTB, TB).transpose(0, 2, 1, 3)
    )

    def keyside(src):
        a = np.zeros((KDIM, nkp), wdt)
        a[:, : len(idx)] = src[b].T[:, idx]
        a = kmajor(a, 6)  # [128, 6, nkp]
        return np.ascontiguousarray(
            a.reshape(128, 6, nkb, 512).transpose(0, 2, 1, 3)
        )

    xk = keyside(key)
    xv = keyside(value)
    mbias = np.full((nkeys,), NEG, np.float32)
    mbias[: len(idx)] = 0.0
    mbias = np.ascontiguousarray(mbias.reshape(nkt, 128).T)
    return {"xq": xq, "xk": xk, "xv": xv, "mbias": mbias, "idx": idx}


def kernel(
    query, key, value, Wq, bq, Wk, bk, Wv, bv, Wo, bo, query_mask, key_mask
):
    global LAST_EXEC_NS, LAST_TRACE_DIR
    from concourse.bass_utils import run_bass_kernel_spmd

    query = np.asarray(query, dtype=np.float32)
    key = np.asarray(key, dtype=np.float32)
    value = np.asarray(value, dtype=np.float32)
    Wq = np.asarray(Wq, dtype=np.float32)
    Wk = np.asarray(Wk, dtype=np.float32)
    Wv = np.asarray(Wv, dtype=np.float32)
    Wo = np.asarray(Wo, dtype=np.float32)
    bq = np.asarray(bq, dtype=np.float32)
    bk = np.asarray(bk, dtype=np.float32)
    bv = np.asarray(bv, dtype=np.float32)
    bo = np.asarray(bo, dtype=np.float32)
    qm = np.asarray(query_mask)
    km = np.asarray(key_mask)

    # host-side key compaction (query_mask masks the KEY axis, globally
    # per batch)
    keep = [np.flatnonzero(qm[b] != 0) for b in range(B)]
    nkeep = max((len(k) for k in keep), default=0)
    nkt = max(1, math.ceil(nkeep / 128))
    nkb = (nkt * 128 + 511) // 512

    with_bv = bool(np.any(bv))
    ck = (nkt, with_bv, BF16, INTERLEAVE, OLD_NORM, WARMUP, MUL_GP)
    if ck not in _CACHE:
        _CACHE[ck] = _build(nkt, with_bv, BF16)
    nc = _CACHE[ck]

    wdt = ml_dtypes.bfloat16 if BF16 else np.float32

    def arr_kmajor(a, ktiles):  # [dim, n] -> [128, ktiles, n]
        return np.ascontiguousarray(
            a.reshape(ktiles, 128, a.shape[1]).transpose(1, 0, 2)
        ).astype(wdt)

    batches = [
        _prep_batch(b, query, key, value, qm, nkt, nkb, wdt) for b in range(B)
    ]

    in_maps = []
    for c in range(N_CORES):
        b, hg = c // HG, c % HG
        hs = hg * GH
        bb = batches[b]
        bqk = np.empty((128, 4), np.float32)
        bqk[:, 0] = bq[hs : hs + 128]
        bqk[:, 1] = bq[hs + 128 : hs + 256]
        bqk[:, 2] = bk[hs : hs + 128]
        bqk[:, 3] = bk[hs + 128 : hs + 256]
        m = {
            "xq": bb["xq"],
            "xk": bb["xk"],
            "xv": bb["xv"],
            "wq": arr_kmajor(Wq[:, hs : hs + GH], 8),
            "wk": arr_kmajor(Wk[:, hs : hs + GH], 6),
            "wv": arr_kmajor(Wv[:, hs : hs + GH], 6),
            "wo": arr_kmajor(Wo[hs : hs + GH, :], 2),
            "mbias": bb["mbias"],
            "bqk": bqk,
        }
        if with_bv:
            bvt = np.empty((128, 2), np.float32)
            bvt[:, 0] = bv[hs : hs + 128]
            bvt[:, 1] = bv[hs + 128 : hs + 256]
            m["bv"] = bvt
        in_maps.append(m)

    kwargs = {}
    if PROFILE:
        import tempfile

        LAST_TRACE_DIR = tempfile.mkdtemp(prefix="bass_trace_")
        kwargs = {"trace": True, "tmpdir": LAST_TRACE_DIR}
    res = run_bass_kernel_spmd(nc, in_maps, list(range(N_CORES)), **kwargs)
    LAST_EXEC_NS = res.exec_time_ns

    out = np.zeros((B, LQ, QDIM), np.float32)
    for c in range(N_CORES):
        out[c // HG] += np.asarray(res.results[c]["outp"]).astype(np.float32)
    out += bo[None, None, :]
    for b in range(B):
        if len(keep[b]) == 0:
            # all keys masked: reference softmax is NaN everywhere
            out[b] = np.nan
    # key_mask masks the QUERY axis in the reference; a zero row makes the
    # whole softmax row -inf -> NaN output for that query position.
    for b in range(B):
        zq = np.flatnonzero(km[b] == 0)
        if len(zq):
            out[b, zq, :] = np.nan
    return out


# revision 19
# speedup vs baseline: 1.0108x; 1.0108x over previous
"""Trainium2 Bass kernel for nn_CrossModalAttention (B=2, LQ=LK=2048,
QDIM=HID=1024, KDIM=VDIM=768, H=16, D=64).

Sharding: 8 cores = 2 batches x 4 head-groups (4 heads each).
Per core: q/k/v projections column-sliced over HID, attention for its 4
heads, row-parallel partial of the out-projection. Host sums the 4
partials per batch (the row-parallel unshard) and adds bo.

Device dataflow (per core), matmuls in bf16 (~4e-3 rel):
  - host passes query/key/value[b] transposed and K/V key-compacted
    (query_mask masks the KEY axis globally per batch); all DRAM
    layouts are arranged so every DMA slice is contiguous per
    partition (few descriptors -> cheap on the sync queue)
  - a short dummy-matmul warmup stream runs during the input DMAs so
    the PE HAM clock-gate is already at 8/8 when real work arrives
  - per head pair (row-packed K=64 matmuls via tile_position):
    scoresT [keys, q] -> ACT exp(s/8 + mask_bias) -> PV matmul with a
    ones-augmented V (M=65) giving ctxT and the softmax denominator
  - normalize: evac ctx PSUM (DVE/gpsimd split), reciprocal of the
    denominator row, gpsimd partition-broadcast, one DVE multiply
  - out-projection is interleaved per token-block into the next
    block's attention (kk-major so the ctx stationary is reused);
    bf16 partials stream to DRAM throughout the kernel
"""

import math

import ml_dtypes
import numpy as np

B, LQ, LK = 2, 2048, 2048
QDIM, KDIM, VDIM, HID, H = 1024, 768, 768, 1024, 16
D = HID // H  # 64
HG = 4  # head-groups (cores per batch)
HL = H // HG  # heads per core = 4
GH = HL * D  # per-core hid slice = 256
N_CORES = 8
TB = 512  # token block
NTB = LQ // TB  # 4
NEG = -1.0e30

BF16 = True
INTERLEAVE = True
OLD_NORM = False
MUL_GP = False
WARMUP = True
PROFILE = False
LAST_EXEC_NS = None
LAST_TRACE_DIR = None

_CACHE = {}
_BATCH_CACHE = {}


def _build(nkt: int, with_bv: bool, bf16: bool):
    import concourse.bacc as bacc
    import concourse.mybir as mybir
    import concourse.tile as tile

    nkeys = nkt * 128
    nkb = (nkeys + 511) // 512  # 512-key blocks
    kbs = [min(512, nkeys - kb * 512) for kb in range(nkb)]

    f32 = mybir.dt.float32
    f32r = mybir.dt.bfloat16 if bf16 else mybir.dt.float32r
    odt = mybir.dt.bfloat16 if bf16 else f32
    Exp = mybir.ActivationFunctionType.Exp

    nc = bacc.Bacc(
        "TRN2", target_bir_lowering=False, debug=False, num_devices=N_CORES
    )

    # DRAM tensors (per-core shapes); layouts chosen so each DMA slice is
    # contiguous per partition
    XQ = nc.dram_tensor("xq", [128, NTB, 8, TB], f32r, kind="ExternalInput").ap()
    XK = nc.dram_tensor("xk", [128, nkb, 6, 512], f32r, kind="ExternalInput").ap()
    XV = nc.dram_tensor("xv", [128, nkb, 6, 512], f32r, kind="ExternalInput").ap()
    WQ = nc.dram_tensor("wq", [128, 8, GH], f32r, kind="ExternalInput").ap()
    WK = nc.dram_tensor("wk", [128, 6, GH], f32r, kind="ExternalInput").ap()
    WV = nc.dram_tensor("wv", [128, 6, GH], f32r, kind="ExternalInput").ap()
    WO = nc.dram_tensor("wo", [128, 2, QDIM], f32r, kind="ExternalInput").ap()
    MB = nc.dram_tensor("mbias", [128, nkt], f32, kind="ExternalInput").ap()
    BQ = nc.dram_tensor("bqk", [128, 4], f32, kind="ExternalInput").ap()
    BV = None
    if with_bv:
        BV = nc.dram_tensor("bv", [128, 2], f32, kind="ExternalInput").ap()
    OUT = nc.dram_tensor("outp", [LQ, QDIM], odt, kind="ExternalOutput").ap()

    with tile.TileContext(nc) as tc:
        with (
            tc.tile_pool(name="consts", bufs=1) as consts,
            tc.tile_pool(name="resid", bufs=1) as resid,
            tc.tile_pool(name="xs", bufs=2) as xs,
            tc.tile_pool(name="probs", bufs=4) as probs_pool,
            tc.tile_pool(name="norm", bufs=3) as norm_pool,
            tc.tile_pool(name="outs", bufs=4) as outs_pool,
            tc.tile_pool(name="ps", bufs=2, space="PSUM") as ps,
        ):
            # ---- PE warmup: dummy matmuls issued before any data lands so
            # the HAM clock-gate reaches 8/8 while the input DMAs stream ----
            wu = consts.tile([128, 512], f32r)
            nc.vector.memset(wu, 0.0)
            wu_ps2 = None
            if WARMUP:
                wu_ps = ps.tile([128, 512], f32, tag="proj", name="warm")
                wu_ps2 = ps.tile([128, 512], f32, tag="proj", name="warm2")
                for _ in range(12):
                    nc.tensor.matmul(wu_ps, wu[:, 0:128], wu, start=True, stop=True)

            # ---- constants / weights ----
            # weights go on the gpsimd SWDGE ring so the big input streams
            # (sync HWDGE ring) aren't serialized behind them
            wq_sb = consts.tile([128, 8, GH], f32r)
            wk_sb = consts.tile([128, 6, GH], f32r)
            wv_sb = consts.tile([128, 6, GH], f32r)
            wo_sb = consts.tile([128, 2, QDIM], f32r)
            mb_sb = consts.tile([128, nkt], f32)
            bqk_sb = consts.tile([128, 4], f32)
            nc.gpsimd.dma_start(out=wq_sb, in_=WQ)
            nc.scalar.dma_start(out=bqk_sb, in_=BQ)
            nc.scalar.dma_start(out=mb_sb, in_=MB)
            nc.gpsimd.dma_start(out=wk_sb, in_=WK)
            nc.gpsimd.dma_start(out=wv_sb, in_=WV)
            nc.gpsimd.dma_start(out=wo_sb, in_=WO)
            bv_sb = None
            if with_bv:
                bv_sb = consts.tile([128, 2], f32)
                nc.gpsimd.dma_start(out=bv_sb, in_=BV)

            # ---- residents ----
            # qT tiles double as ctxT tiles later (WAR handled by Tile)
            qT = [resid.tile([128, LQ], f32r, tag=f"qT{p}", name=f"qT{p}") for p in range(2)]
            kT = [resid.tile([128, nkeys], f32r, tag=f"kT{p}", name=f"kT{p}") for p in range(2)]
            v_sb = resid.tile([128, nkt, HL, D + 1], f32r)
            # ones columns for the denominator rows: fill the whole tile,
            # the v-projection copies then overwrite the [., ., ., 0:D] part
            if bf16:
                nc.vector.memset(v_sb, 1.0)
            else:
                nc.vector.memset(v_sb[:, :, :, :].bitcast(f32), 1.0)

            # ---- k projection (per key-block, so attention can begin
            # after kb0) ----
            def emit_xk(kb_i):
                xk_t = xs.tile([128, 6, 512], f32r, tag="xk", name="xk_t", bufs=3)
                nc.sync.dma_start(out=xk_t, in_=XK[:, kb_i])
                return xk_t

            def emit_kproj_m(kb_i, m, xk_t):
                kbw = kbs[kb_i]
                s0 = kb_i * 512
                ps_t = ps.tile([128, 512], f32, tag="proj", name="kp_ps")
                for k in range(6):
                    nc.tensor.matmul(
                        ps_t[:, :kbw],
                        wk_sb[:, k, m * 128 : (m + 1) * 128],
                        xk_t[:, k, :kbw],
                        start=(k == 0),
                        stop=(k == 5),
                    )
                nc.vector.tensor_scalar_add(
                    kT[m][:, s0 : s0 + kbw],
                    ps_t[:, :kbw],
                    bqk_sb[:, 2 + m : 3 + m],
                )

            # ---- v projection ----
            vproj_state = {}

            def emit_vproj_kt(kt):
                kb_i = kt // 4
                sub = kt % 4
                if sub == 0 and kb_i not in vproj_state:
                    xv_t = xs.tile([128, 6, 512], f32r, tag="xv", name="xv_t")
                    nc.scalar.dma_start(out=xv_t, in_=XV[:, kb_i])
                    vproj_state[kb_i] = xv_t
                xv_t = vproj_state[kb_i]
                ps_t = ps.tile([128, 512], f32, tag="proj", name="vp_ps")
                for k in range(6):
                    nc.tensor.matmul(
                        ps_t[:, :GH],
                        xv_t[:, k, sub * 128 : (sub + 1) * 128],
                        wv_sb[:, k, :],
                        start=(k == 0),
                        stop=(k == 5),
                    )
                nc.vector.tensor_copy(
                    v_sb[:, kt, :, 0:D],
                    ps_t[:, :GH].rearrange("p (h d) -> p h d", h=HL),
                )

            def emit_xq(tb, eng=None):
                xq_t = xs.tile([128, 8, TB], f32r, tag="xq", name="xq_t")
                (eng or nc.sync).dma_start(out=xq_t, in_=XQ[:, tb])
                return xq_t

            def emit_qproj_m(tb, m, xq_t):
                t0 = tb * TB
                ps_t = ps.tile([128, 512], f32, tag="proj", name="qp_ps")
                for k in range(8):
                    nc.tensor.matmul(
                        ps_t,
                        wq_sb[:, k, m * 128 : (m + 1) * 128],
                        xq_t[:, k, :],
                        start=(k == 0),
                        stop=(k == 7),
                    )
                nc.vector.tensor_scalar_add(
                    qT[m][:, t0 : t0 + TB],
                    ps_t,
                    bqk_sb[:, m : m + 1],
                )

            def emit_scores(p, tb, kt, prtag, prbufs):
                t0 = tb * TB
                k0 = kt * 128
                sc = ps.tile([128, 2, TB], f32, tag="sc", name="sc")
                for hh in range(2):
                    nc.tensor.matmul(
                        sc[:, hh, :],
                        kT[p][hh * 64 : hh * 64 + 64, k0 : k0 + 128],
                        qT[p][hh * 64 : hh * 64 + 64, t0 : t0 + TB],
                        start=True,
                        stop=True,
                        tile_position=(hh * 64, 0),
                    )
                pr = probs_pool.tile(
                    [128, 2, TB], f32r, tag=prtag, name="pr", bufs=prbufs
                )
                nc.scalar.activation(
                    pr, sc, Exp, bias=mb_sb[:, kt : kt + 1], scale=0.125
                )
                return pr

            def emit_pv(p, tb, kt, pr, ctx_ps):
                for hh in range(2):
                    nc.tensor.matmul(
                        ctx_ps[hh],
                        v_sb[:, kt, 2 * p + hh, :],
                        pr[:, hh, :],
                        start=(kt == 0),
                        stop=(kt == nkt - 1),
                    )

            def emit_normalize(p, tb, ctx_ps, last=False):
                t0 = tb * TB

                def finish(hh, src, mul_eng):
                    dcp = norm_pool.tile([1, TB], f32, tag="dcp", name="dcp")
                    nc.vector.tensor_copy(dcp, src[D : D + 1, :])
                    rec = norm_pool.tile([1, TB], f32, tag="rec", name="rec")
                    nc.vector.reciprocal_approx_fast(out=rec, in_=dcp)
                    rbc = norm_pool.tile([D, TB], f32, tag="rbc", name="rbc")
                    nc.gpsimd.partition_broadcast(rbc, rec)
                    dst = qT[p][hh * 64 : hh * 64 + 64, t0 : t0 + TB]
                    mul_eng.tensor_mul(dst, src[0:D, :], rbc)
                    if with_bv:
                        nc.vector.tensor_scalar_add(
                            dst, dst, bv_sb[64 * hh : 64 * hh + 64, p : p + 1]
                        )

                if OLD_NORM:
                    evac = []
                    for hh in range(2):
                        ctmp = norm_pool.tile([D, TB], f32, tag="ctmp", name="ctmp")
                        nc.vector.tensor_copy(ctmp, ctx_ps[hh][0:D, :])
                        dcp = norm_pool.tile([1, TB], f32, tag="dcp", name="dcp")
                        nc.vector.tensor_copy(dcp, ctx_ps[hh][D : D + 1, :])
                        evac.append((ctmp, dcp))
                    for hh in range(2):
                        ctmp, dcp = evac[hh]
                        rbc = norm_pool.tile([D, TB], f32, tag="rbc", name="rbc")
                        nc.gpsimd.partition_broadcast(rbc, dcp)
                        rec = norm_pool.tile([D, TB], f32, tag="rec2", name="rec2")
                        nc.vector.reciprocal_approx_fast(out=rec, in_=rbc)
                        dst = qT[p][hh * 64 : hh * 64 + 64, tb * TB : (tb + 1) * TB]
                        nc.vector.tensor_mul(dst, ctmp, rec)
                        if with_bv:
                            nc.vector.tensor_scalar_add(
                                dst, dst, bv_sb[64 * hh : 64 * hh + 64, p : p + 1]
                            )
                elif last:
                    for hh in range(2):
                        finish(hh, ctx_ps[hh], nc.vector)
                else:
                    # evac both ctx tiles first so the next block's PV
                    # matmuls get their PSUM slots fast; the multiplies are
                    # SBUF-only and go to gpsimd to keep DVE light
                    evs = []
                    for hh in range(2):
                        ev = norm_pool.tile(
                            [D + 1, TB], f32, tag=f"ev{hh}", name="ev"
                        )
                        nc.vector.tensor_copy(ev, ctx_ps[hh])
                        evs.append(ev)
                    for hh in range(2):
                        finish(hh, evs[hh], nc.gpsimd if MUL_GP else nc.vector)

            def emit_attn(p, tb, thunks=(), last=False):
                thunks = list(thunks)
                ctx_ps = [
                    ps.tile([D + 1, TB], f32, tag="ctx", name=f"ctx{p}_{tb}_{i}")
                    for i in range(2)
                ]
                prs = [emit_scores(p, tb, 0, "pr", 4)]
                for kt in range(nkt):
                    if kt + 1 < nkt:
                        prs.append(emit_scores(p, tb, kt + 1, "pr", 4))
                    emit_pv(p, tb, kt, prs[kt], ctx_ps)
                    if kt < len(thunks):
                        thunks[kt]()
                for th in thunks[nkt:]:
                    th()
                emit_normalize(p, tb, ctx_ps, last=last)

            def qproj_thunks(tb):
                # 7 small thunks: the xq DMA, then 3+3+2 matmuls per
                # m-half chained into one psum accumulation
                cell = {}

                def start(tb=tb):
                    cell["xq"] = emit_xq(tb, nc.scalar if tb == 1 else nc.sync)

                def chunk(m, k0, k1, fin, tb=tb):
                    if k0 == 0:
                        cell[m] = ps.tile(
                            [128, 512], f32, tag="proj", name="qp_ps"
                        )
                    ps_t = cell[m]
                    for k in range(k0, k1):
                        nc.tensor.matmul(
                            ps_t,
                            wq_sb[:, k, m * 128 : (m + 1) * 128],
                            cell["xq"][:, k, :],
                            start=(k == 0),
                            stop=(k == 7),
                        )
                    if fin:
                        nc.vector.tensor_scalar_add(
                            qT[m][:, tb * TB : (tb + 1) * TB],
                            ps_t,
                            bqk_sb[:, m : m + 1],
                        )

                out = [start]
                for m in range(2):
                    out += [
                        lambda m=m: chunk(m, 0, 3, False),
                        lambda m=m: chunk(m, 3, 6, False),
                        lambda m=m: chunk(m, 6, 8, True),
                    ]
                return out

            def emit_outproj_tt(tt):
                # kk-major: one ctx stationary load feeds both nh halves
                ps0 = ps.tile([128, 512], f32, tag="proj", name="op0")
                ps1 = ps.tile([128, 512], f32, tag="proj", name="op1")
                for kk in range(2):
                    lhs = qT[kk][:, tt * 128 : (tt + 1) * 128]
                    nc.tensor.matmul(
                        ps0, lhs, wo_sb[:, kk, 0:512],
                        start=(kk == 0), stop=(kk == 1),
                    )
                    nc.tensor.matmul(
                        ps1, lhs, wo_sb[:, kk, 512:1024],
                        start=(kk == 0), stop=(kk == 1),
                    )
                for nh, pst in ((0, ps0), (1, ps1)):
                    o_sb = outs_pool.tile([128, 512], odt, tag="osb", name="o_sb")
                    if nh == 0:
                        nc.scalar.copy(o_sb, pst)
                    else:
                        nc.vector.tensor_copy(o_sb, pst)
                    oeng = nc.gpsimd if (tt + nh) % 2 == 0 else nc.sync
                    oeng.dma_start(
                        out=OUT[
                            tt * 128 : (tt + 1) * 128,
                            nh * 512 : (nh + 1) * 512,
                        ],
                        in_=o_sb,
                    )

            def outproj_thunks(tb):
                return [
                    (lambda tt=tt: emit_outproj_tt(tt))
                    for tt in range(4 * tb, 4 * tb + 4)
                ]

            # ---- emission schedule ----
            # sync-queue order = need order: xk0, xq0, xk1, xk2, ...
            xk_ts = [emit_xk(0)]
            xq0 = emit_xq(0)
            xk_ts += [emit_xk(kb_i) for kb_i in range(1, nkb)]
            emit_kproj_m(0, 0, xk_ts[0])
            emit_kproj_m(0, 1, xk_ts[0])
            emit_qproj_m(0, 0, xq0)
            emit_qproj_m(0, 1, xq0)
            kt_kb0 = min(4, nkt)
            prs0 = [emit_scores(0, 0, kt, "pr0", nkt) for kt in range(kt_kb0)]
            for kb_i in range(1, nkb):
                emit_kproj_m(kb_i, 0, xk_ts[kb_i])
                emit_kproj_m(kb_i, 1, xk_ts[kb_i])
            prs0 += [
                emit_scores(0, 0, kt, "pr0", nkt) for kt in range(kt_kb0, nkt)
            ]
            for kt in range(nkt):
                emit_vproj_kt(kt)
            ctx0 = [
                ps.tile([D + 1, TB], f32, tag="ctx", name=f"ctx00_{i}")
                for i in range(2)
            ]
            for kt in range(nkt):
                emit_pv(0, 0, kt, prs0[kt], ctx0)
            emit_normalize(0, 0, ctx0)
            if INTERLEAVE:
                emit_attn(1, 0, thunks=qproj_thunks(1))
                for tb in range(1, NTB):
                    op = outproj_thunks(tb - 1)
                    emit_attn(0, tb, thunks=op[:2])
                    th = op[2:]
                    if tb + 1 < NTB:
                        th = th + qproj_thunks(tb + 1)
                    emit_attn(1, tb, thunks=th, last=(tb == NTB - 1))
                for _ in range(10):
                    nc.tensor.matmul(
                        wu_ps2, wu[:, 0:128], wu, start=True, stop=True
                    )
                for tt in range(4 * (NTB - 1), 4 * NTB):
                    emit_outproj_tt(tt)
            else:
                emit_attn(1, 0)
                for tb in range(1, NTB):
                    xq_t = emit_xq(tb)
                    emit_qproj_m(tb, 0, xq_t)
                    emit_attn(0, tb)
                    emit_qproj_m(tb, 1, xq_t)
                    emit_attn(1, tb)
                for tt in range(4 * NTB):
                    emit_outproj_tt(tt)

    nc.compile()
    return nc


def _prep_batch(b, query, key, value, qm, nkt, nkb, wdt):
    """Per-batch input arrays (shared by the 4 cores of a batch)."""
    nkeys = nkt * 128
    nkp = nkb * 512
    idx = np.flatnonzero(qm[b] != 0)

    def kmajor(a, ktiles):  # [dim, n] -> [128, ktiles, n]
        return np.ascontiguousarray(
            a.reshape(ktiles, 128, a.shape[1]).transpose(1, 0, 2)
        )

    xq = kmajor(np.ascontiguousarray(query[b].T).astype(wdt), 8)
    # [128, 8, LQ] -> [128, NTB, 8, TB]
    xq = np.ascontiguousarray(
        xq.reshape(128, 8, NTB, TB).transpose(0, 2, 1, 3)
    )

    def keyside(src):
        a = np.zeros((KDIM, nkp), wdt)
        a[:, : len(idx)] = src[b].T[:, idx]
        a = kmajor(a, 6)  # [128, 6, nkp]
        return np.ascontiguousarray(
            a.reshape(128, 6, nkb, 512).transpose(0, 2, 1, 3)
        )

    xk = keyside(key)
    xv = keyside(value)
    mbias = np.full((nkeys,), NEG, np.float32)
    mbias[: len(idx)] = 0.0
    mbias = np.ascontiguousarray(mbias.reshape(nkt, 128).T)
    return {"xq": xq, "xk": xk, "xv": xv, "mbias": mbias, "idx": idx}


def kernel(
    query, key, value, Wq, bq, Wk, bk, Wv, bv, Wo, bo, query_mask, key_mask
):
    global LAST_EXEC_NS, LAST_TRACE_DIR
    from concourse.bass_utils import run_bass_kernel_spmd

    query = np.asarray(query, dtype=np.float32)
    key = np.asarray(key, dtype=np.float32)
    value = np.asarray(value, dtype=np.float32)
    Wq = np.asarray(Wq, dtype=np.float32)
    Wk = np.asarray(Wk, dtype=np.float32)
    Wv = np.asarray(Wv, dtype=np.float32)
    Wo = np.asarray(Wo, dtype=np.float32)
    bq = np.asarray(bq, dtype=np.float32)
    bk = np.asarray(bk, dtype=np.float32)
    bv = np.asarray(bv, dtype=np.float32)
    bo = np.asarray(bo, dtype=np.float32)
    qm = np.asarray(query_mask)
    km = np.asarray(key_mask)

    # host-side key compaction (query_mask masks the KEY axis, globally
    # per batch)
    keep = [np.flatnonzero(qm[b] != 0) for b in range(B)]
    nkeep = max((len(k) for k in keep), default=0)
    nkt = max(1, math.ceil(nkeep / 128))
    nkb = (nkt * 128 + 511) // 512

    with_bv = bool(np.any(bv))
    ck = (nkt, with_bv, BF16, INTERLEAVE, OLD_NORM, WARMUP, MUL_GP)
    if ck not in _CACHE:
        _CACHE[ck] = _build(nkt, with_bv, BF16)
    nc = _CACHE[ck]

    wdt = ml_dtypes.bfloat16 if BF16 else np.float32

    def arr_kmajor(a, ktiles):  # [dim, n] -> [128, ktiles, n]
        return np.ascontiguousarray(
            a.reshape(ktiles, 128, a.shape[1]).transpose(1, 0, 2)
        ).astype(wdt)

    batches = [
        _prep_batch(b, query, key, value, qm, nkt, nkb, wdt) for b in range(B)
    ]

    in_maps = []
    for c in range(N_CORES):
        b, hg = c // HG, c % HG
        hs = hg * GH
        bb = batches[b]
        bqk = np.empty((128, 4), np.float32)
        bqk[:, 0] = bq[hs : hs + 128]
        bqk[:, 1] = bq[hs + 128 : hs + 256]
        bqk[:, 2] = bk[hs : hs + 128]
        bqk[:, 3] = bk[hs + 128 : hs + 256]
        m = {
            "xq": bb["xq"],
            "xk": bb["xk"],
            "xv": bb["xv"],
            "wq": arr_kmajor(Wq[:, hs : hs + GH], 8),
            "wk": arr_kmajor(Wk[:, hs : hs + GH], 6),
            "wv": arr_kmajor(Wv[:, hs : hs + GH], 6),
            "wo": arr_kmajor(Wo[hs : hs + GH, :], 2),
            "mbias": bb["mbias"],
            "bqk": bqk,
        }
        if with_bv:
            bvt = np.empty((128, 2), np.float32)
            bvt[:, 0] = bv[hs : hs + 128]
            bvt[:, 1] = bv[hs + 128 : hs + 256]
            m["bv"] = bvt
        in_maps.append(m)

    kwargs = {}
    if PROFILE:
        import tempfile

        LAST_TRACE_DIR = tempfile.mkdtemp(prefix="bass_trace_")
        kwargs = {"trace": True, "tmpdir": LAST_TRACE_DIR}
    res = run_bass_kernel_spmd(nc, in_maps, list(range(N_CORES)), **kwargs)
    LAST_EXEC_NS = res.exec_time_ns

    out = np.zeros((B, LQ, QDIM), np.float32)
    for c in range(N_CORES):
        out[c // HG] += np.asarray(res.results[c]["outp"]).astype(np.float32)
    out += bo[None, None, :]
    for b in range(B):
        if len(keep[b]) == 0:
            # all keys masked: reference softmax is NaN everywhere
            out[b] = np.nan
    # key_mask masks the QUERY axis in the reference; a zero row makes the
    # whole softmax row -inf -> NaN output for that query position.
    for b in range(B):
        zq = np.flatnonzero(km[b] == 0)
        if len(zq):
            out[b, zq, :] = np.nan
    return out
